# revision 20
# baseline (speedup 1.0000x reference)
"""Trainium2 Bass kernel for windowed multi-head attention with relative
position bias (FAXModule / SwissCheese).

Reference computation (per batch element b of 64):
    tokens = x[b].reshape(512, 625).T                  # (n, d)
    qkv    = tokens @ w_qkv;  q,k,v heads of dim 64, 8 heads
    sim    = (q * dh^-0.5) @ k^T + bias[idx]           # (8, 625, 625)
    out    = softmax(sim) @ v  -> concat -> @ w_out -> (512, 625)

Sharding: pure data parallel, 8 batches per NeuronCore; weights and the
(batch-independent) relative-position bias tables are replicated.

On-device layout (per batch, token dim padded 625->626 for fp32r's
even-free-dim ISA restriction; the pad token gets x=0 and bias=-30 so its
attention weight is exp(-34) ~ 0):
    qk^T  [1024, 626]  = W_qk^T @ x_b    (q rows pre-scaled by dh^-0.5, fp32r)
    v     [626, 512]   = x_b^T @ W_v     (j on partitions -> PV lhsT, fp16)
    sim^T [j, i] per head, 2 heads row-packed on the PE (K=64 each, fp32r);
    rel-pos bias added by an identity-matmul accumulation into the same
    PSUM bank (bf16); exp(x-4) on ScalarE PSUM->fp16 SBUF; PV col-packs the
    head pair (fp16) with a [ones] block computing softmax denominators
    partition-aligned; normalize = reciprocal_approx_fast + tensor_mul;
    out-proj W_out^T @ cat (fp32r) back to [d, n] layout.
"""

import sys
from contextlib import ExitStack

sys.path.insert(0, "/opt/trn_rl_repo")

import numpy as np
import ml_dtypes

import concourse.bass as bass
import concourse.bacc as bacc
import concourse.tile as tile
import concourse.mybir as mybir
from concourse.bass_utils import run_bass_kernel_spmd

B = 64
D = 512
WS = 25
N = WS * WS          # 625 tokens
NP = 626             # padded token dim (fp32r needs even free dims)
DH = 64
H = 8                # heads
NCORES = 8
BPC = B // NCORES    # 8 batches per core
JCH = [(0, 128), (128, 128), (256, 128), (384, 128), (512, 114)]
NJ = len(JCH)
ICHUNKS = [(0, 320), (320, 306)]   # even, >=256 for f32r full rate

F32 = mybir.dt.float32
F32R = mybir.dt.float32r
BF16 = mybir.dt.bfloat16
FP16 = mybir.dt.float16
EXP = mybir.ActivationFunctionType.Exp
EXP_SHIFT = -4.0     # exp(x-4): keeps attn weights in fp16 range


def build_tile_kernel(tc, ctx, x_d, wqk_d, wv_d, wo_d, bias_d, ones_d,
                      out_d, bpc=BPC):
    nc = tc.nc

    # ---- constant pools (resident for the whole kernel) ----
    cpool = ctx.enter_context(tc.tile_pool(name="const", bufs=1))

    wqk_sb = []
    for k in range(4):
        t = cpool.tile([128, 2 * D], F32R, tag=f"wqk{k}", name=f"wqk{k}")
        nc.sync.dma_start(out=t[:, :], in_=wqk_d[128 * k:128 * (k + 1), :])
        wqk_sb.append(t)
    wv_sb = []
    for k in range(4):
        t = cpool.tile([128, D], F32R, tag=f"wv{k}", name=f"wv{k}")
        nc.sync.dma_start(out=t[:, :], in_=wv_d[128 * k:128 * (k + 1), :])
        wv_sb.append(t)
    wo_sb = []
    for k in range(4):
        t = cpool.tile([128, D], F32R, tag=f"wo{k}", name=f"wo{k}")
        nc.sync.dma_start(out=t[:, :], in_=wo_d[128 * k:128 * (k + 1), :])
        wo_sb.append(t)
    ones_t = cpool.tile([128, 128], FP16, tag="ones", name="ones_t")
    nc.sync.dma_start(out=ones_t[:, :], in_=ones_d[:, :])
    shift_t = cpool.tile([128, 1], F32, tag="shift", name="shift_t")
    nc.vector.memset(shift_t[:, :], EXP_SHIFT)

    # E = exp(bias) tables: [head][jchunk] -> [jlen, 626] fp16 (~6.3 MB);
    # attn = exp(sim - 4) * E, so the rel-pos bias costs one fp16 2x-mode
    # DVE multiply instead of PE identity-matmuls.
    e_sb = []
    for h in range(H):
        row = []
        for (j0, jl) in JCH:
            t = cpool.tile([128, NP], FP16, tag=f"ebias{h}_{j0}",
                           name=f"ebias{h}_{j0}")
            nc.scalar.dma_start(out=t[0:jl, :], in_=bias_d[h, j0:j0 + jl, :])
            row.append(t)
        e_sb.append(row)

    # ---- working pools ----
    xp = ctx.enter_context(tc.tile_pool(name="xp", bufs=2))
    qkp = ctx.enter_context(tc.tile_pool(name="qkp", bufs=1))
    vp = ctx.enter_context(tc.tile_pool(name="vp", bufs=2))
    ap_pool = ctx.enter_context(tc.tile_pool(name="attn", bufs=14))
    ocp = ctx.enter_context(tc.tile_pool(name="ocp", bufs=2))
    fp = ctx.enter_context(tc.tile_pool(name="fp", bufs=4))
    rp = ctx.enter_context(tc.tile_pool(name="rp", bufs=4))

    pproj = ctx.enter_context(tc.tile_pool(name="pproj", bufs=2, space="PSUM"))
    psim = ctx.enter_context(tc.tile_pool(name="psim", bufs=3, space="PSUM"))
    ppv = ctx.enter_context(tc.tile_pool(name="ppv", bufs=3, space="PSUM"))

    for b in range(bpc):
        # ---------- load x_b ----------
        x_sb = []
        for k in range(4):
            t = xp.tile([128, NP], F32R, tag=f"x{k}", name=f"x{b}_{k}")
            nc.gpsimd.dma_start(out=t[:, :], in_=x_d[b, 128 * k:128 * (k + 1), :])
            x_sb.append(t)

        # ---------- qk projection: qk^T = W_qk^T @ x_b  [1024, 626] ----------
        qk_sb = []
        for m in range(8):
            qs = qkp.tile([128, NP], F32R, tag=f"qk{m}", name=f"qk{b}_{m}")
            for (i0, il) in ICHUNKS:
                ps = pproj.tile([128, 512], F32, tag="proj",
                                name=f"psqk{b}_{m}_{i0}")
                for k in range(4):
                    nc.tensor.matmul(
                        ps[:, 0:il],
                        wqk_sb[k][:, 128 * m:128 * (m + 1)],
                        x_sb[k][:, i0:i0 + il],
                        start=(k == 0),
                        stop=(k == 3),
                    )
                nc.vector.tensor_copy(qs[:, i0:i0 + il], ps[:, 0:il])
            qk_sb.append(qs)

        # ---------- v projection: v = x_b^T @ W_v [626, 512] (fp16) ----------
        v_sb = []
        for (j0, jl) in JCH:
            vs = vp.tile([128, D], FP16, tag=f"v{j0}", name=f"v{b}_{j0}")
            ps = pproj.tile([128, 512], F32, tag="proj", name=f"psv{b}_{j0}")
            for k in range(4):
                nc.tensor.matmul(
                    ps[0:jl, 0:D],
                    x_sb[k][:, j0:j0 + jl],
                    wv_sb[k][:, :],
                    start=(k == 0),
                    stop=(k == 3),
                )
            nc.vector.tensor_copy(vs[0:jl, :], ps[0:jl, 0:D])
            v_sb.append(vs)

        # ---------- attention ----------
        oc_sb = []
        for hp in range(4):
            t = ocp.tile([128, NP], F32R, tag=f"oc{hp}", name=f"oc{b}_{hp}")
            oc_sb.append(t)

        for (i0, il) in ICHUNKS:
            for hp in range(4):
                hA, hB = 2 * hp, 2 * hp + 1
                q_t = qk_sb[hp]
                k_t = qk_sb[4 + hp]
                attA = []
                attB = []
                for jc, (j0, jl) in enumerate(JCH):
                    psA = psim.tile([128, 512], F32, tag="sim",
                                    name=f"sA{b}_{i0}_{hp}_{j0}")
                    psB = psim.tile([128, 512], F32, tag="sim",
                                    name=f"sB{b}_{i0}_{hp}_{j0}")
                    # sim^T = k_h^T.T @ q_h^T ; 2 heads row-packed (K=64 each)
                    nc.tensor.matmul(
                        psA[0:jl, 0:il],
                        k_t[0:64, j0:j0 + jl],
                        q_t[0:64, i0:i0 + il],
                        start=True, stop=True,
                    )
                    nc.tensor.matmul(
                        psB[0:jl, 0:il],
                        k_t[64:128, j0:j0 + jl],
                        q_t[64:128, i0:i0 + il],
                        start=True, stop=True,
                    )
                    aA = ap_pool.tile([128, 320], FP16, tag="attn",
                                      name=f"aA{b}_{i0}_{hp}_{j0}")
                    aB = ap_pool.tile([128, 320], FP16, tag="attn",
                                      name=f"aB{b}_{i0}_{hp}_{j0}")
                    nc.scalar.activation(aA[0:jl, 0:il], psA[0:jl, 0:il],
                                         EXP, bias=shift_t[0:jl, 0:1])
                    nc.scalar.activation(aB[0:jl, 0:il], psB[0:jl, 0:il],
                                         EXP, bias=shift_t[0:jl, 0:1])
                    nc.vector.tensor_mul(
                        aA[0:jl, 0:il], aA[0:jl, 0:il],
                        e_sb[hA][jc][0:jl, i0:i0 + il],
                    )
                    nc.vector.tensor_mul(
                        aB[0:jl, 0:il], aB[0:jl, 0:il],
                        e_sb[hB][jc][0:jl, i0:i0 + il],
                    )
                    attA.append(aA)
                    attB.append(aB)

                # PV (fp16): out rows = [headA(0:64) | headB(64:128)],
                # col-packed; denominators into a second bank, same packing.
                pout = ppv.tile([128, 512], F32, tag="pv",
                                name=f"po{b}_{i0}_{hp}")
                psum = ppv.tile([128, 512], F32, tag="pv",
                                name=f"pd{b}_{i0}_{hp}")
                for jc, (j0, jl) in enumerate(JCH):
                    nc.tensor.matmul(
                        pout[0:64, 0:il],
                        v_sb[jc][0:jl, DH * hA:DH * hA + DH],
                        attA[jc][0:jl, 0:il],
                        start=(jc == 0), stop=(jc == NJ - 1),
                        tile_position=(0, 0),
                        skip_group_check=True,
                    )
                    nc.tensor.matmul(
                        pout[64:128, 0:il],
                        v_sb[jc][0:jl, DH * hB:DH * hB + DH],
                        attB[jc][0:jl, 0:il],
                        start=(jc == 0), stop=(jc == NJ - 1),
                        tile_position=(0, 64),
                        skip_group_check=True,
                    )
                    nc.tensor.matmul(
                        psum[0:64, 0:il],
                        ones_t[0:jl, 0:64],
                        attA[jc][0:jl, 0:il],
                        start=(jc == 0), stop=(jc == NJ - 1),
                        tile_position=(0, 0),
                        skip_group_check=True,
                    )
                    nc.tensor.matmul(
                        psum[64:128, 0:il],
                        ones_t[0:jl, 64:128],
                        attB[jc][0:jl, 0:il],
                        start=(jc == 0), stop=(jc == NJ - 1),
                        tile_position=(0, 64),
                        skip_group_check=True,
                    )
                rc = rp.tile([128, 320], F32, tag="recip",
                             name=f"rc{b}_{i0}_{hp}")
                nc.vector.reciprocal_approx_fast(rc[:, 0:il], psum[:, 0:il])
                nc.vector.tensor_mul(
                    oc_sb[hp][:, i0:i0 + il], pout[:, 0:il], rc[:, 0:il]
                )

        # ---------- output projection: final^T = W_out^T @ cat ----------
        for m in range(4):
            for (i0, il) in ICHUNKS:
                ps = pproj.tile([128, 512], F32, tag="proj",
                                name=f"pso{b}_{m}_{i0}")
                for p in range(4):
                    nc.tensor.matmul(
                        ps[:, 0:il],
                        wo_sb[p][:, 128 * m:128 * (m + 1)],
                        oc_sb[p][:, i0:i0 + il],
                        start=(p == 0),
                        stop=(p == 3),
                    )
                ft = fp.tile([128, 320], F32, tag="final",
                             name=f"ft{b}_{m}_{i0}")
                nc.vector.tensor_copy(ft[:, 0:il], ps[:, 0:il])
                olen = min(il, N - i0)
                nc.sync.dma_start(
                    out=out_d[b, 128 * m:128 * (m + 1), i0:i0 + olen],
                    in_=ft[:, 0:olen],
                )


def build_nc(bpc=BPC):
    nc = bacc.Bacc("TRN2", target_bir_lowering=False, debug=False)
    x_d = nc.dram_tensor("x", [bpc, D, NP], F32R, kind="ExternalInput").ap()
    wqk_d = nc.dram_tensor("wqk", [D, 2 * D], F32R, kind="ExternalInput").ap()
    wv_d = nc.dram_tensor("wv", [D, D], F32R, kind="ExternalInput").ap()
    wo_d = nc.dram_tensor("wo", [D, D], F32R, kind="ExternalInput").ap()
    bias_d = nc.dram_tensor("biasT", [H, NP, NP], FP16,
                            kind="ExternalInput").ap()
    ones_d = nc.dram_tensor("ones", [128, 128], FP16,
                            kind="ExternalInput").ap()
    out_d = nc.dram_tensor("out", [bpc, D, N], F32, kind="ExternalOutput").ap()
    with tile.TileContext(nc) as tc:
        with ExitStack() as ctx:
            build_tile_kernel(tc, ctx, x_d, wqk_d, wv_d, wo_d, bias_d,
                              ones_d, out_d, bpc=bpc)
    nc.compile()
    return nc


_NC = None


def _round_f32r(a):
    """Round fp32 to the fp32r grid (11 explicit mantissa bits), RNE."""
    u = np.ascontiguousarray(a, dtype=np.float32).view(np.uint32)
    r = (u + np.uint32(0x7FF) + ((u >> np.uint32(12)) & np.uint32(1))) \
        & np.uint32(0xFFFFF000)
    return r.view(np.float32)


def _prep_inputs(x, w_qkv, w_out, rel_pos_bias, rel_pos_indices):
    x = np.asarray(x, dtype=np.float32).reshape(B, D, N)
    w_qkv = np.asarray(w_qkv, dtype=np.float32)
    w_out = np.asarray(w_out, dtype=np.float32)
    rel_pos_bias = np.asarray(rel_pos_bias, dtype=np.float32)
    rel_pos_indices = np.asarray(rel_pos_indices)

    scale = DH ** -0.5
    wqk = _round_f32r(np.concatenate(
        [w_qkv[:, :D] * scale, w_qkv[:, D:2 * D]], axis=1
    ).astype(np.float32))
    wv = _round_f32r(w_qkv[:, 2 * D:])
    w_out = _round_f32r(w_out)
    xp_ = np.zeros((B, D, NP), np.float32)
    xp_[:, :, :N] = x
    xp_ = _round_f32r(xp_)
    # E[m, j, i] = exp(bias[i, j, m]); pad KEY row (j=625) -> 0 so real
    # queries ignore the fake token; pad QUERY col (i=625) -> 1 so its
    # softmax denominator stays healthy (output discarded).
    bias_arr = rel_pos_bias[rel_pos_indices]            # (625, 625, 8) f32
    biasT = np.zeros((H, NP, NP), np.float32)
    biasT[:, :N, :N] = np.exp(bias_arr.transpose(2, 1, 0))
    biasT[:, :N, N] = 1.0
    biasT = biasT.astype(np.float16)
    ones = np.ones((128, 128), dtype=np.float16)

    in_maps = []
    for c in range(NCORES):
        in_maps.append({
            "x": np.ascontiguousarray(xp_[c * BPC:(c + 1) * BPC]),
            "wqk": wqk,
            "wv": wv,
            "wo": np.ascontiguousarray(w_out),
            "biasT": biasT,
            "ones": ones,
        })
    return in_maps


def _run(in_maps, trace=False, tmpdir=None):
    global _NC
    if _NC is None:
        _NC = build_nc()
    kw = {}
    if tmpdir is not None:
        kw["tmpdir"] = tmpdir
    res = run_bass_kernel_spmd(
        _NC, in_maps, core_ids=list(range(NCORES)), trace=trace, **kw
    )
    out = np.concatenate([res.results[c]["out"] for c in range(NCORES)], axis=0)
    return out.reshape(B, D, WS, WS), res


def kernel(x, w_qkv, w_out, rel_pos_bias, rel_pos_indices):
    in_maps = _prep_inputs(x, w_qkv, w_out, rel_pos_bias, rel_pos_indices)
    out, _ = _run(in_maps, trace=False)
    return out


class _Runner:
    """Holds a jitted 8-core SPMD executable for repeated timed dispatch."""

    def __init__(self, nc, in_maps):
        import jax
        from jax.sharding import Mesh, PartitionSpec, NamedSharding
        from jax.experimental.shard_map import shard_map
        from concourse import bass2jax
        import concourse.mybir as mb
        self._jax = jax
        bass2jax.install_neuronx_cc_hook()
        self._setup(nc, in_maps, jax, Mesh, PartitionSpec, NamedSharding,
                    shard_map, bass2jax, mb)

    def _setup(self, nc, in_maps, jax, Mesh, PartitionSpec, NamedSharding,
               shard_map, bass2jax, mb):

        in_names, out_names, out_avals, zero_outs = [], [], [], []
        for alloc in nc.m.functions[0].allocations:
            if not isinstance(alloc, mb.MemoryLocationSet):
                continue
            name = alloc.memorylocations[0].name
            if alloc.kind == "ExternalInput":
                if nc.partition_id_tensor is None or \
                        name != nc.partition_id_tensor.name:
                    in_names.append(name)
            elif alloc.kind == "ExternalOutput":
                out_names.append(name)
                shape = tuple(alloc.tensor_shape)
                dtype = mb.dt.np(alloc.dtype)
                out_avals.append(jax.core.ShapedArray(shape, dtype))
                zero_outs.append(np.zeros(shape, dtype))
        n_params = len(in_names)
        pname = nc.partition_id_tensor.name if nc.partition_id_tensor else None

        def _body(*args):
            operands = list(args)
            if pname is not None:
                operands.append(bass2jax.partition_id_tensor())
            outs = bass2jax._bass_exec_p.bind(
                *operands,
                out_avals=tuple(out_avals),
                in_names=tuple(in_names + out_names
                               + ([pname] if pname else [])),
                out_names=tuple(out_names),
                lowering_input_output_aliases=(),
                sim_require_finite=True,
                sim_require_nnan=True,
                nc=nc,
            )
            return tuple(outs)

        devices = jax.devices()[:NCORES]
        mesh = Mesh(np.asarray(devices), ("core",))
        spec = PartitionSpec("core")
        n_out = len(out_names)
        self.sharded = jax.jit(
            shard_map(
                _body, mesh=mesh,
                in_specs=(spec,) * (n_params + n_out),
                out_specs=(spec,) * n_out,
                check_rep=False,
            ),
            keep_unused=True,
        )
        sh = NamedSharding(mesh, spec)
        self.concat_in = [
            jax.device_put(
                np.concatenate(
                    [np.asarray(in_maps[c][nm]) for c in range(NCORES)], 0
                ),
                sh,
            )
            for nm in in_names
        ]
        self.concat_zeros = [
            jax.device_put(
                np.zeros((NCORES * z.shape[0], *z.shape[1:]), z.dtype), sh)
            for z in zero_outs
        ]
        self.out_names = out_names
        self.last = None

    def run(self):
        self.last = self._jax.block_until_ready(
            self.sharded(*self.concat_in, *self.concat_zeros))
        return self.last

    def timed(self, nrep):
        import time
        t0 = time.perf_counter()
        rs = [self.sharded(*self.concat_in, *self.concat_zeros)
              for _ in range(nrep)]
        self._jax.block_until_ready(rs)
        return time.perf_counter() - t0

    def slope(self, iters=5, lo_n=8, hi_n=264):
        import statistics
        lo, hi = [], []
        self.timed(4)
        for _ in range(iters):
            lo.append(self.timed(lo_n))
            hi.append(self.timed(hi_n))
        return (statistics.median(hi) - statistics.median(lo)) / (hi_n - lo_n)

    def output(self, name, shape):
        oi = self.out_names.index(name)
        return np.asarray(self.last[oi]).reshape(shape)


def kernel_timed(x, w_qkv, w_out, rel_pos_bias, rel_pos_indices, iters=5):
    """Correctness + steady-state timing.

    Returns (out, t_kernel_ns, info): t_kernel_ns = 8 * marginal-batch time
    measured as the slope between a 16-batch and 8-batch variant of the
    same kernel (launch/dispatch overheads cancel exactly)."""
    global _NC
    if _NC is None:
        _NC = build_nc()
    in_maps = _prep_inputs(x, w_qkv, w_out, rel_pos_bias, rel_pos_indices)
    r8 = _Runner(_NC, in_maps)
    r8.run()
    out = r8.output("out", (NCORES, BPC, D, N)).reshape(B, D, WS, WS)

    # dispatch submission floor is ~1.2 ms/call; use 16- vs 24-batch
    # variants (both device-bound) so the floor cancels in the delta.
    nc16 = build_nc(bpc=2 * BPC)
    in_maps16 = [
        {**m, "x": np.concatenate([m["x"], m["x"]], axis=0)} for m in in_maps
    ]
    r16 = _Runner(nc16, in_maps16)
    r16.run()
    slope16 = r16.slope(iters=iters)

    nc24 = build_nc(bpc=3 * BPC)
    in_maps24 = [
        {**m, "x": np.concatenate([m["x"]] * 3, axis=0)} for m in in_maps
    ]
    r24 = _Runner(nc24, in_maps24)
    r24.run()
    slope24 = r24.slope(iters=iters)

    t_batch = (slope24 - slope16) / BPC
    t_kernel = t_batch * BPC
    info = {"slope16_ns": int(slope16 * 1e9),
            "slope24_ns": int(slope24 * 1e9),
            "t_batch_ns": int(t_batch * 1e9)}
    return out, int(t_kernel * 1e9), info
